# revision 26
# baseline (speedup 1.0000x reference)
"""Trainium2 Bass kernel for nn_BackflowNet (gnn_message_passing).

Computation per walker b (B=256, N=64, D=3):
    r_ij = x_i - x_j ; feats = [x_i, x_j, r, |r|, |r|^2]  (r folded into W1)
    m_ij = silu(silu(feats @ W1 + b1) @ W2 + b2)          (128-dim messages)
    m_i  = sum_{j != i} m_ij
    out  = tanh(psi([x, m_i])) * bf_scale                 (3-layer MLP psi)

Sharding: pure data parallel over B across 8 cores (32 walkers/core),
params replicated.

Design (three-engine silu split; baseline was ACT-bound at ~304us):
  - Message path in fp16: full-speed PE matmuls, DVE 2x/4x modes.
  - Walkers in groups of 4; [128, 2048] feature supertiles; L1 uses
    16-tile (32x32) PE packing; L2 full-width.  ACT1 (silu1) runs
    entirely on the ACT engine (2048-col ACTIVATEs from PSUM).
  - silu2 is SPLIT: most sblocks on ACT; DVE_SB sblocks run a fp16
    bit-trick sigmoid on the DVE (Schraudolph exp via fp->int16
    convert + magic-constant reciprocal + 1 Newton step; constants
    tuned against the real z2 distribution, rms err ~3e-3/element).
    The Pool engine converts those sblocks' PSUM fp32 -> SBUF fp16.
  - The masked j-sum (fold) is a pairwise-add tree: on Pool for
    POOL_FOLD sblocks, on DVE for the rest; diagonal zeroed by Pool
    memsets.
  - r1=sqrt(r2) via DVE Quake rsqrt (+1 Newton); startup emits build
    piece 0 + its scatters first so the first L1 starts early; a dummy
    activation preloads the SILU table.
  - psi MLP tail runs in two 1024-col halves, pipelined.
"""

import numpy as np

B, N, D = 256, 64, 3
NCORES = 8
BW = B // NCORES        # walkers per core
MSG_H = 128
HID = 128
NG = BW // 4            # walker groups of 4 per core
NJC = 32                # j-values per chunk
NCHUNK = 2              # chunks per walker (j in [0,32) and [32,64))
GC_COLS = 2048          # pair columns per (group, chunk) supertile

# ---- silu2 engine split tunables ----
# global sblock id gsb = 4*unit_idx + s, unit_idx = c*NG + g
DVE_SB = frozenset(range(9, 61, 3))
# fold tree on Pool for these sblocks (rest on DVE)
POOL_FOLD = frozenset(g for g in range(64) if g % 8 in (3, 5, 6))
# DVE bit-trick sigmoid constants (tuned offline vs the z2 distribution)
S_ES = 1466.6667        # exp scale
S_EO = 13344.0          # exp offset
S_DS = 3.71940          # d = y*S_DS + 1
S_MG = 0x77f6           # reciprocal magic
S_CLO, S_CHI = -10.5, 8.5


def build_program(bw=BW):
    import concourse.bass as bass
    import concourse.bacc as bacc
    import concourse.tile as tile
    import concourse.mybir as mybir

    F32 = mybir.dt.float32
    F16 = mybir.dt.float16
    AF = mybir.ActivationFunctionType
    AO = mybir.AluOpType
    U16, I16 = mybir.dt.uint16, mybir.dt.int16
    npart = bw * N          # particle rows per core for the psi stage (2048)

    nc = bacc.Bacc("TRN2", target_bir_lowering=False, debug=False)

    xh_h = nc.dram_tensor("xh", [bw, D, N], F16, kind="ExternalInput")
    w1p_h = nc.dram_tensor("w1p", [128, MSG_H], F16, kind="ExternalInput")
    b1_h = nc.dram_tensor("b1", [MSG_H, 1], F32, kind="ExternalInput")
    w2_h = nc.dram_tensor("w2", [MSG_H, MSG_H], F16, kind="ExternalInput")
    b2_h = nc.dram_tensor("b2", [MSG_H, 1], F32, kind="ExternalInput")
    pw1m_h = nc.dram_tensor("pw1m", [MSG_H, HID], F16, kind="ExternalInput")
    pw1x_h = nc.dram_tensor("pw1x", [D, HID], F16, kind="ExternalInput")
    pb1_h = nc.dram_tensor("pb1", [HID, 1], F32, kind="ExternalInput")
    pw2_h = nc.dram_tensor("pw2", [HID, HID], F16, kind="ExternalInput")
    pb2_h = nc.dram_tensor("pb2", [HID, 1], F32, kind="ExternalInput")
    pw3_h = nc.dram_tensor("pw3", [HID, D], F16, kind="ExternalInput")
    pb3_h = nc.dram_tensor("pb3", [D, 1], F32, kind="ExternalInput")
    sc_h = nc.dram_tensor("sc", [D, 1], F32, kind="ExternalInput")
    out_h = nc.dram_tensor("out", [bw, D, N], F16, kind="ExternalOutput")
    import os
    dbg = os.environ.get("BASS_DBG", "")
    dbg_h = None
    if dbg:
        dbg_h = nc.dram_tensor("dbg", [MSG_H, 8192], F16, kind="ExternalOutput")

    with tile.TileContext(nc) as tc:
        with (
            tc.tile_pool(name="consts", bufs=1) as consts,
            tc.tile_pool(name="build", bufs=1) as build,
            tc.tile_pool(name="rkeep", bufs=1) as rkeep,
            tc.tile_pool(name="feat", bufs=3) as featp,
            tc.tile_pool(name="hpool", bufs=2) as hpool,
            tc.tile_pool(name="mpool", bufs=2) as mpool,
            tc.tile_pool(name="fold", bufs=2) as fold,
            tc.tile_pool(name="sig", bufs=1) as sigp,
            tc.tile_pool(name="xzp", bufs=2) as xzp,
            tc.tile_pool(name="tail", bufs=2) as tailp,
            tc.tile_pool(name="ps", bufs=1, space="PSUM") as ps,
        ):
            # ---- Fbuf zeroing on Pool (idle at start); rows 8..31 of each
            # 32-row quadrant must stay 0 for the K=32 L1 contraction ----
            F_bufs = []
            for fi in range(4):
                Fb = featp.tile([128, GC_COLS], F16, name=f"Fbuf{fi}")
                nc.gpsimd.memset(Fb, 0.0)
                F_bufs.append(Fb)

            # dummy activation: pull the SILU ACT_TABLE_LOAD off the
            # critical path (table loads with the first activation)
            dummy_t = consts.tile([MSG_H, 1], F32)
            nc.vector.memset(dummy_t, 0.0)
            nc.scalar.activation(dummy_t, dummy_t, AF.Silu, bias=0.0)

            # ---- critical constants first (serial sync DMA queue) ----
            w1p_t = consts.tile([128, MSG_H], F16)
            nc.sync.dma_start(out=w1p_t, in_=w1p_h.ap())
            b1_t = consts.tile([MSG_H, 1], F32)
            nc.sync.dma_start(out=b1_t, in_=b1_h.ap())
            # m_i accumulator, col = w*64 + i
            stash_t = consts.tile([MSG_H, npart], F16)

            # ---- stacked r build (fp16), partition = 32*d + w ----
            r2bf = rkeep.tile([32, 2 * GC_COLS], F16, name="r2bf")
            r1bf = rkeep.tile([32, 2 * GC_COLS], F16, name="r1bf")
            xjk = [None, None]

            def emit_build_dmas(c):
                XI = build.tile([96, GC_COLS], F16, tag="XI",
                                name=f"XI{c}")
                XJc = build.tile([96, NJC], F16, tag="XJc",
                                 name=f"XJc{c}")
                for d in range(D):
                    nc.sync.dma_start(
                        out=XI[32 * d:32 * d + 32, :].rearrange(
                            "p (j i) -> p j i", i=N),
                        in_=bass.AP(xh_h, d * N, [[N * D, 32], [0, NJC], [1, N]]),
                    )
                    nc.sync.dma_start(
                        out=XJc[32 * d:32 * d + 32, :],
                        in_=bass.AP(xh_h, d * N + NJC * c,
                                    [[N * D, 32], [1, NJC]]),
                    )
                return XI, XJc

            def emit_build_piece(c, XI, XJc, sl, jsl, xjf, tb, tc_, t1, t2):
                # one j-piece of the r build: r2, r1 (Quake rsqrt + Newton)
                xjbc = XJc[:, jsl].unsqueeze(2).broadcast_to(
                    [96, jsl.stop - jsl.start, N])
                nc.vector.tensor_copy(
                    xjf[:, sl].rearrange("p (j i) -> p j i", i=N), xjbc)
                nc.vector.tensor_sub(
                    XI[:, sl].rearrange("p (j i) -> p j i", i=N),
                    XI[:, sl].rearrange("p (j i) -> p j i", i=N), xjbc)
                nc.vector.tensor_mul(XI[:, sl], XI[:, sl], XI[:, sl])
                nc.sync.dma_start(out=tb[:, sl], in_=XI[32:64, sl])
                nc.sync.dma_start(out=tc_[:, sl], in_=XI[64:96, sl])
                nc.vector.tensor_add(tb[:, sl], XI[0:32, sl], tb[:, sl])
                r2c = r2bf[:, GC_COLS * c:GC_COLS * (c + 1)][:, sl]
                r1c = r1bf[:, GC_COLS * c:GC_COLS * (c + 1)][:, sl]
                nc.vector.tensor_add(r2c, tb[:, sl], tc_[:, sl])  # r2
                nc.vector.tensor_scalar(r2c, r2c, 1e-4, None, AO.add)
                nc.vector.tensor_scalar(
                    t1[:, sl].bitcast(U16), r2c.bitcast(U16),
                    1, None, AO.logical_shift_right)
                nc.vector.tensor_scalar(
                    t1[:, sl].bitcast(I16), t1[:, sl].bitcast(I16),
                    0x59b8, -1, AO.subtract, AO.mult)
                nc.vector.tensor_mul(t2[:, sl], t1[:, sl], t1[:, sl])
                nc.vector.tensor_mul(t2[:, sl], t2[:, sl], r2c)
                nc.vector.tensor_scalar(t2[:, sl], t2[:, sl], -0.5, 1.5,
                                        AO.mult, AO.add)
                nc.vector.tensor_mul(t2[:, sl], t1[:, sl], t2[:, sl])
                nc.vector.tensor_mul(r1c, r2c, t2[:, sl])

            def mk_build_tiles(c):
                # scratch tags shared across chunks (chunk 1 reuses chunk 0's)
                xjf = rkeep.tile([96, GC_COLS], F16, name=f"xjf{c}")
                tb = build.tile([32, GC_COLS], F16, tag="tb", name=f"tb{c}")
                tc_ = build.tile([32, GC_COLS], F16, tag="tc", name=f"tc{c}")
                t1 = build.tile([32, GC_COLS], F16, tag="qt1", name=f"qt1{c}")
                t2 = build.tile([32, GC_COLS], F16, tag="qt2", name=f"qt2{c}")
                xjk[c] = xjf
                return xjf, tb, tc_, t1, t2

            def emit_feat_xi_dmas(g, F):
                for d in range(D):
                    nc.sync.dma_start(
                        out=F[d:128:32, :].rearrange("q (j i) -> q j i", i=N),
                        in_=bass.AP(xh_h, 4 * g * N * D + d * N,
                                    [[N * D, 4], [0, NJC], [1, N]]),
                    )

            def emit_feat_scatters(g, c, F, sl):
                for d in range(D):
                    nc.sync.dma_start(
                        out=F[3 + d:128:32, sl],
                        in_=xjk[c][32 * d + 4 * g:32 * d + 4 * g + 4, sl],
                    )
                nc.sync.dma_start(
                    out=F[6:128:32, sl],
                    in_=r2bf[:, GC_COLS * c:GC_COLS * (c + 1)][4 * g:4 * g + 4, sl])
                nc.sync.dma_start(
                    out=F[7:128:32, sl],
                    in_=r1bf[:, GC_COLS * c:GC_COLS * (c + 1)][4 * g:4 * g + 4, sl])

            # ---- startup: build chunk 0 piece 0, then its feat DMAs ----
            XI0, XJc0 = emit_build_dmas(0)
            F0 = F_bufs[0]
            emit_feat_xi_dmas(0, F0)      # xi rows: no build dependency
            bt0 = mk_build_tiles(0)
            PW0 = GC_COLS // 4
            emit_build_piece(0, XI0, XJc0, slice(0, PW0), slice(0, 8), *bt0)
            emit_feat_scatters(0, 0, F0, slice(0, PW0))
            # w2/b2 needed by L2 of sblock 0 (~15us in)
            w2_t = consts.tile([MSG_H, MSG_H], F16)
            nc.sync.dma_start(out=w2_t, in_=w2_h.ap())
            b2_t = consts.tile([MSG_H, 1], F32)
            nc.sync.dma_start(out=b2_t, in_=b2_h.ap())
            for p in range(1, 4):
                emit_build_piece(0, XI0, XJc0,
                                 slice(PW0 * p, PW0 * (p + 1)),
                                 slice(8 * p, 8 * (p + 1)), *bt0)
                emit_feat_scatters(0, 0, F0, slice(PW0 * p, PW0 * (p + 1)))

            # ---- main stream over 16 (group, chunk) units, 4 sblocks each;
            # ACT2/L2 lag ACT1/L1 by one sblock ----
            gcs = [(g, c) for c in range(NCHUNK) for g in range(NG)]

            def emit_feat_dmas(g, c, idx):
                # called at unit idx-1's top (one-unit prefetch)
                F = F_bufs[idx % 4]
                emit_feat_xi_dmas(g, F)
                emit_feat_scatters(g, c, F, slice(0, GC_COLS))
                return F

            def emit_l1(F, s):
                psA = ps.tile([MSG_H, GC_COLS], F32, tag="A")
                for q in range(4):
                    for mq in range(4):
                        nc.tensor.matmul(
                            psA[32 * mq:32 * (mq + 1), 512 * q:512 * (q + 1)],
                            w1p_t[32 * q:32 * q + 32, 32 * mq:32 * (mq + 1)],
                            F[32 * q:32 * q + 32, 512 * s:512 * (s + 1)],
                            start=True, stop=True,
                            tile_position=(32 * q, 32 * mq),
                        )
                return psA

            # DVE bit-trick silu tiles
            sA = sigp.tile([MSG_H, GC_COLS], F16, name="sigA")
            sB = sigp.tile([MSG_H, GC_COLS], F16, name="sigB")
            sC = sigp.tile([MSG_H, GC_COLS], F16, name="sigC")

            def emit_silu2_head(psB):
                # Single PSUM reader: xz = fp16(psB + b2) -- the bias that
                # ACT2 would have applied.  Emitted with high priority so
                # the Vector queue reaches it promptly (otherwise psB blocks
                # the next sblock's L2 behind the multi-us chain backlog).
                xz = xzp.tile([MSG_H, GC_COLS], F16, tag="xz")
                with tc.high_priority(offset=260):
                    nc.vector.tensor_scalar(xz, psB, b2_t, None, AO.add)
                return xz

            def emit_silu2_chain(xz, ms):
                nc.vector.tensor_scalar(sA, xz, S_CLO, S_CHI, AO.max, AO.min)
                nc.vector.tensor_scalar(sB.bitcast(I16), sA, -S_ES, S_EO,
                                        AO.mult, AO.add)
                nc.vector.tensor_scalar(sA, sB, S_DS, 1.0, AO.mult, AO.add)
                nc.vector.tensor_scalar(sB.bitcast(I16), sA.bitcast(I16),
                                        S_MG, -1, AO.subtract, AO.mult)
                nc.vector.tensor_mul(sC, sA, sB)        # t = d*r0
                nc.vector.tensor_scalar(sC, sC, -1.0, 2.0, AO.mult, AO.add)
                nc.vector.tensor_mul(sB, sB, sC)        # r = r0*(2-t)
                nc.vector.tensor_mul(ms, xz, sB)        # m = x*r

            def emit_l2_matmuls(h, s):
                psB = ps.tile([MSG_H, GC_COLS], F32, tag="B")
                for k in range(4):
                    nc.tensor.matmul(
                        psB[:, 512 * k:512 * (k + 1)],
                        w2_t,
                        h[:, 2048 * s + 512 * k:2048 * s + 512 * (k + 1)],
                        start=True, stop=True,
                    )
                return psB

            def emit_fold_piece(g, c, s, m, v3, gsb):
                # zero self-messages: within sblock s cols = 512q + 64jj + i,
                # diag at i = 32c + 8s + jj -> col = 512q + 65jj + 32c + 8s
                ms = m[:, 2048 * s:2048 * (s + 1)]
                v = ms.rearrange("p (q b) -> p q b", q=4)
                nc.gpsimd.memset(v[:, :, 32 * c + 8 * s::65], 0.0)
                eng = nc.gpsimd if gsb in POOL_FOLD else nc.vector
                # 3-level pairwise tree over the 8 j's, in place in m
                m3 = ms.rearrange("p (q j i) -> p q j i", q=4, j=8)
                eng.tensor_add(m3[:, :, 0:4, :], m3[:, :, 0:4, :],
                               m3[:, :, 4:8, :])
                eng.tensor_add(m3[:, :, 0:2, :], m3[:, :, 0:2, :],
                               m3[:, :, 2:4, :])
                eng.tensor_add(
                    v3[:, 256 * s:256 * (s + 1)].rearrange(
                        "p (q i) -> p q i", q=4),
                    m3[:, :, 0, :], m3[:, :, 1, :])

            def emit_fold_final(g, c, v3):
                # cross-sblock + stash adds on Pool (otherwise idle)
                ta = fold.tile([MSG_H, 256], F16, tag="ta")
                nc.gpsimd.tensor_add(ta, v3[:, 0:256], v3[:, 256:512])
                tb2 = fold.tile([MSG_H, 256], F16, tag="tb2")
                nc.gpsimd.tensor_add(tb2, v3[:, 512:768], v3[:, 768:1024])
                sl = stash_t[:, 256 * g:256 * (g + 1)]
                if c == 0:
                    nc.gpsimd.tensor_add(sl, ta, tb2)
                else:
                    tc2 = fold.tile([MSG_H, 256], F16, tag="tc2")
                    nc.gpsimd.tensor_add(tc2, ta, tb2)
                    nc.gpsimd.tensor_add(sl, sl, tc2)

            psi_consts = {}

            def emit_psi_consts():
                pw1m_t = consts.tile([MSG_H, HID], F16)
                nc.sync.dma_start(out=pw1m_t, in_=pw1m_h.ap())
                pw1x_t = consts.tile([D, HID], F16)
                nc.sync.dma_start(out=pw1x_t, in_=pw1x_h.ap())
                pw2_t = consts.tile([HID, HID], F16)
                nc.sync.dma_start(out=pw2_t, in_=pw2_h.ap())
                pw3_t = consts.tile([HID, D], F16)
                nc.sync.dma_start(out=pw3_t, in_=pw3_h.ap())
                pb1_t = consts.tile([HID, 1], F32)
                nc.sync.dma_start(out=pb1_t, in_=pb1_h.ap())
                pb2_t = consts.tile([HID, 1], F32)
                nc.sync.dma_start(out=pb2_t, in_=pb2_h.ap())
                pb3_t = consts.tile([D, 1], F32)
                nc.sync.dma_start(out=pb3_t, in_=pb3_h.ap())
                sc_t = consts.tile([D, 1], F32)
                nc.sync.dma_start(out=sc_t, in_=sc_h.ap())
                xT_t = consts.tile([D, npart], F16)
                nc.sync.dma_start(
                    out=xT_t,
                    in_=bass.AP(xh_h, 0, [[N, D], [N * D, bw], [1, N]]),
                )
                psi_consts.update(pw1m=pw1m_t, pw1x=pw1x_t, pw2=pw2_t,
                                  pw3=pw3_t, pb1=pb1_t, pb2=pb2_t, pb3=pb3_t,
                                  sc=sc_t, xT=xT_t)

            unit_info = {}  # (g, c) -> {'v3': tile, 'done': set()}
            pending = []    # deferred DVE chains: (g, c, s, m, xz, slot)

            def fold_and_maybe_final(g, c, s, m, gsb):
                ui = unit_info[(g, c)]
                emit_fold_piece(g, c, s, m, ui['v3'], gsb)
                ui['done'].add(s)
                if len(ui['done']) == 4:
                    emit_fold_final(g, c, ui['v3'])

            def flush_pending(before_slot):
                while pending and pending[0][5] < before_slot:
                    fg, fc, fs, fm, fxz, _ = pending.pop(0)
                    emit_silu2_chain(fxz, fm[:, 2048 * fs:2048 * (fs + 1)])
                    fold_and_maybe_final(fg, fc, fs, fm,
                                         4 * (fc * NG + fg) + fs)

            prev = None  # (g, c, h, m, s) of the lagging sblock stream
            slot = 0
            for idx, (g, c) in enumerate(gcs):
                if idx == 1:
                    # chunk-1 r build mid-stream (DVE has slack early)
                    XI1, XJc1 = emit_build_dmas(1)
                    bt1 = mk_build_tiles(1)
                    for p in range(2):
                        emit_build_piece(1, XI1, XJc1,
                                         slice(1024 * p, 1024 * (p + 1)),
                                         slice(16 * p, 16 * (p + 1)), *bt1)
                if idx == 4:
                    emit_psi_consts()
                # one-unit feature prefetch: emit unit idx+1's DMAs now
                if idx + 1 < len(gcs):
                    ng_, nc_ = gcs[idx + 1]
                    emit_feat_dmas(ng_, nc_, idx + 1)
                F = F_bufs[idx % 4]
                h = hpool.tile([MSG_H, 4 * GC_COLS], F16, tag="h",
                               name=f"h_{g}_{c}")
                m = mpool.tile([MSG_H, 4 * GC_COLS], F16, tag="m",
                               name=f"m_{g}_{c}")
                v3u = fold.tile([MSG_H, 1024], F16, tag="v3",
                                name=f"v3_{g}_{c}")
                unit_info[(g, c)] = {'v3': v3u, 'done': set()}
                for s in range(4):
                    flush_pending(slot)
                    pl2 = prev
                    psA = emit_l1(F, s)
                    nc.scalar.activation(h[:, 2048 * s:2048 * (s + 1)], psA,
                                         AF.Silu, bias=b1_t, scale=1.0)
                    if pl2 is not None:
                        pg, pc, ph, pm, psb = pl2
                        pgsb = 4 * (pc * NG + pg) + psb
                        psB = emit_l2_matmuls(ph, psb)
                        if pgsb in DVE_SB:
                            xz = emit_silu2_head(psB)
                            pending.append((pg, pc, psb, pm, xz, slot))
                        else:
                            nc.scalar.activation(
                                pm[:, 2048 * psb:2048 * (psb + 1)], psB,
                                AF.Silu, bias=b2_t, scale=1.0)
                            fold_and_maybe_final(pg, pc, psb, pm, pgsb)
                    prev = (g, c, h, m, s)
                    slot += 1
                if dbg == "h0" and (g, c) == (0, 0):
                    nc.sync.dma_start(out=bass.AP(dbg_h, 0, [[8192, MSG_H], [1, 8192]]), in_=h)
            # drain the lagging sblock + deferred chains
            pg, pc, ph, pm, psb = prev
            pgsb = 4 * (pc * NG + pg) + psb
            psB = emit_l2_matmuls(ph, psb)
            if pgsb in DVE_SB:
                xz = emit_silu2_head(psB)
                pending.append((pg, pc, psb, pm, xz, slot))
            else:
                nc.scalar.activation(pm[:, 2048 * psb:2048 * (psb + 1)], psB,
                                     AF.Silu, bias=b2_t, scale=1.0)
                fold_and_maybe_final(pg, pc, psb, pm, pgsb)
            flush_pending(slot + 1)
            if dbg == "m0":
                nc.sync.dma_start(
                    out=bass.AP(dbg_h, 0, [[8192, MSG_H], [1, 8192]]), in_=pm)
            if dbg == "stash":
                nc.sync.dma_start(
                    out=bass.AP(dbg_h, 0, [[8192, MSG_H], [1, 2048]]),
                    in_=stash_t)

            # ---- psi MLP tail, two 1024-col halves pipelined ----
            pc_ = psi_consts
            for hf in range(2):
                sl = slice(1024 * hf, 1024 * (hf + 1))
                u1 = tailp.tile([HID, 1024], F16, tag="u1")
                u2 = tailp.tile([HID, 1024], F16, tag="u2")
                dxs = tailp.tile([D, 1024], F16, tag="dxs")
                psA = ps.tile([HID, 1024], F32, tag="A")
                for s in range(2):
                    ssl = slice(1024 * hf + 512 * s, 1024 * hf + 512 * (s + 1))
                    osl = slice(512 * s, 512 * (s + 1))
                    nc.tensor.matmul(psA[:, osl], pc_["pw1m"], stash_t[:, ssl],
                                     start=True, stop=False)
                    nc.tensor.matmul(psA[:, osl], pc_["pw1x"], pc_["xT"][:, ssl],
                                     start=False, stop=True)
                nc.scalar.activation(u1, psA, AF.Silu, bias=pc_["pb1"], scale=1.0)
                psB = ps.tile([HID, 1024], F32, tag="B")
                for s in range(2):
                    osl = slice(512 * s, 512 * (s + 1))
                    nc.tensor.matmul(psB[:, osl], pc_["pw2"], u1[:, osl],
                                     start=True, stop=True)
                nc.scalar.activation(u2, psB, AF.Silu, bias=pc_["pb2"], scale=1.0)
                psD = ps.tile([D, 1024], F32, tag="A")
                for s in range(2):
                    osl = slice(512 * s, 512 * (s + 1))
                    nc.tensor.matmul(psD[:, osl], pc_["pw3"], u2[:, osl],
                                     start=True, stop=True)
                nc.scalar.activation(dxs, psD, AF.Tanh, bias=pc_["pb3"], scale=1.0)
                nc.vector.tensor_scalar_mul(dxs, dxs, pc_["sc"])
                nc.sync.dma_start(
                    out=bass.AP(out_h, 16 * hf * N * D,
                                [[N, D], [N * D, bw // 2], [1, N]]),
                    in_=dxs.rearrange("p (w i) -> p w i", i=N),
                )

    nc.compile()
    return nc


def host_inputs(x, phi_w1, phi_b1, phi_w2, phi_b2,
                psi_w1, psi_b1, psi_w2, psi_b2, psi_w3, psi_b3, bf_scale,
                bw=BW, ncores=NCORES):
    """Per-core in_maps from the full problem inputs."""
    F16 = np.float16
    x = np.asarray(x, np.float32)
    w1 = np.asarray(phi_w1, np.float64)
    w1p = np.concatenate([
        w1[0:3] + w1[6:9],      # xi rows (r folded in)
        w1[3:6] - w1[6:9],      # xj rows
        w1[10:11],              # r2
        w1[9:10],               # r1
    ], axis=0)
    sc = np.maximum(np.float32(bf_scale), 0.0)
    w1p4 = np.zeros((128, MSG_H), np.float64)
    for q in range(4):
        w1p4[32 * q:32 * q + 8] = w1p
    const = {
        "w1p": w1p4.astype(F16),
        "b1": np.asarray(phi_b1, np.float32).reshape(MSG_H, 1),
        "w2": np.asarray(phi_w2, F16),
        "b2": np.asarray(phi_b2, np.float32).reshape(MSG_H, 1),
        "pw1x": np.ascontiguousarray(np.asarray(psi_w1, F16)[0:3]),
        "pw1m": np.ascontiguousarray(np.asarray(psi_w1, F16)[3:]),
        "pb1": np.asarray(psi_b1, np.float32).reshape(HID, 1),
        "pw2": np.asarray(psi_w2, F16),
        "pb2": np.asarray(psi_b2, np.float32).reshape(HID, 1),
        "pw3": np.asarray(psi_w3, F16),
        "pb3": np.asarray(psi_b3, np.float32).reshape(D, 1),
        "sc": np.full((D, 1), sc, np.float32),
    }
    in_maps = []
    for core in range(ncores):
        xs = np.ascontiguousarray(
            x[core * bw:(core + 1) * bw].transpose(0, 2, 1))
        in_maps.append({"xh": xs.astype(F16), **const})
    return in_maps


_cached_nc = None
LAST_EXEC_NS = None
LAST_PROFILE_JSON = None
LAST_TRACE_PATH = None


def kernel(x, spin, phi_w1, phi_b1, phi_w2, phi_b2,
           psi_w1, psi_b1, psi_w2, psi_b2, psi_w3, psi_b3, bf_scale):
    global _cached_nc
    from concourse.bass_utils import run_bass_kernel_spmd

    if _cached_nc is None:
        _cached_nc = build_program()
    in_maps = host_inputs(x, phi_w1, phi_b1, phi_w2, phi_b2,
                          psi_w1, psi_b1, psi_w2, psi_b2, psi_w3, psi_b3,
                          bf_scale)
    import os
    trace = bool(os.environ.get("BASS_TRACE"))
    res = run_bass_kernel_spmd(_cached_nc, in_maps, core_ids=list(range(NCORES)),
                               trace=trace)
    global LAST_EXEC_NS, LAST_PROFILE_JSON, LAST_TRACE_PATH
    if res.exec_time_ns is not None:
        LAST_EXEC_NS = res.exec_time_ns
    if res.profile_json is not None:
        LAST_PROFILE_JSON = res.profile_json
    if res.instructions_and_trace is not None:
        LAST_TRACE_PATH = res.instructions_and_trace[1]
    out = np.concatenate(
        [r["out"].transpose(0, 2, 1) for r in res.results], axis=0)
    return out.astype(np.float32)


# revision 32
# speedup vs baseline: 1.3140x; 1.3140x over previous
"""Trainium2 Bass kernel for nn_BackflowNet (gnn_message_passing).

Computation per walker b (B=256, N=64, D=3):
    r_ij = x_i - x_j ; feats = [x_i, x_j, r, |r|, |r|^2]  (r folded into W1)
    m_ij = silu(silu(feats @ W1 + b1) @ W2 + b2)          (128-dim messages)
    m_i  = sum_{j != i} m_ij
    out  = tanh(psi([x, m_i])) * bf_scale                 (3-layer MLP psi)

Sharding: pure data parallel over B across 8 cores (32 walkers/core),
params replicated.

Design (three-engine silu split; baseline was ACT-bound at ~304us):
  - Message path in fp16: full-speed PE matmuls, DVE 2x/4x modes.
  - Walkers in groups of 4; [128, 2048] feature supertiles; L1 uses
    16-tile (32x32) PE packing; L2 full-width.  ACT1 (silu1) runs
    entirely on the ACT engine (2048-col ACTIVATEs from PSUM).
  - silu2 is SPLIT: most sblocks on ACT; DVE_SB sblocks run a fp16
    bit-trick sigmoid on the DVE (Schraudolph exp via fp->int16
    convert + magic-constant reciprocal + 1 Newton step; constants
    tuned against the real z2 distribution, rms err ~3e-3/element).
    The Pool engine converts those sblocks' PSUM fp32 -> SBUF fp16.
  - The masked j-sum (fold) is a pairwise-add tree: on Pool for
    POOL_FOLD sblocks, on DVE for the rest; diagonal zeroed by Pool
    memsets.
  - r1=sqrt(r2) via DVE Quake rsqrt (+1 Newton); startup emits build
    piece 0 + its scatters first so the first L1 starts early; a dummy
    activation preloads the SILU table.
  - psi MLP tail runs in two 1024-col halves, pipelined.
"""

import numpy as np

B, N, D = 256, 64, 3
NCORES = 8
BW = B // NCORES        # walkers per core
MSG_H = 128
HID = 128
NG = BW // 4            # walker groups of 4 per core
NJC = 32                # j-values per chunk
NCHUNK = 2              # chunks per walker (j in [0,32) and [32,64))
GC_COLS = 2048          # pair columns per (group, chunk) supertile

# ---- silu2 engine split tunables ----
# global sblock id gsb = 4*unit_idx + s, unit_idx = c*NG + g
# DVE silu2 offload disabled: each offloaded sblock's psB WAR must be
# served by the (congested) Vector queue before the next sblock's L2 can
# start -- the resulting ~7us bubble exceeds the ~2us ACT saving.
DVE_SB = frozenset()
POOL_FOLD = frozenset()
# DVE bit-trick sigmoid constants (tuned offline vs the z2 distribution)
S_ES = 1466.6667        # exp scale
S_EO = 13344.0          # exp offset
S_DS = 3.71940          # d = y*S_DS + 1
S_MG = 0x77f6           # reciprocal magic
S_CLO, S_CHI = -10.5, 8.5


def build_program(bw=BW):
    import concourse.bass as bass
    import concourse.bacc as bacc
    import concourse.tile as tile
    import concourse.mybir as mybir

    F32 = mybir.dt.float32
    F16 = mybir.dt.float16
    AF = mybir.ActivationFunctionType
    AO = mybir.AluOpType
    U16, I16 = mybir.dt.uint16, mybir.dt.int16
    npart = bw * N          # particle rows per core for the psi stage (2048)

    nc = bacc.Bacc("TRN2", target_bir_lowering=False, debug=False)

    xh_h = nc.dram_tensor("xh", [bw, D, N], F16, kind="ExternalInput")
    w1p_h = nc.dram_tensor("w1p", [128, MSG_H], F16, kind="ExternalInput")
    b1_h = nc.dram_tensor("b1", [MSG_H, 1], F32, kind="ExternalInput")
    w2_h = nc.dram_tensor("w2", [MSG_H, MSG_H], F16, kind="ExternalInput")
    b2_h = nc.dram_tensor("b2", [MSG_H, 1], F32, kind="ExternalInput")
    pw1m_h = nc.dram_tensor("pw1m", [MSG_H, HID], F16, kind="ExternalInput")
    pw1x_h = nc.dram_tensor("pw1x", [D, HID], F16, kind="ExternalInput")
    pb1_h = nc.dram_tensor("pb1", [HID, 1], F32, kind="ExternalInput")
    pw2_h = nc.dram_tensor("pw2", [HID, HID], F16, kind="ExternalInput")
    pb2_h = nc.dram_tensor("pb2", [HID, 1], F32, kind="ExternalInput")
    pw3_h = nc.dram_tensor("pw3", [HID, D], F16, kind="ExternalInput")
    pb3_h = nc.dram_tensor("pb3", [D, 1], F32, kind="ExternalInput")
    sc_h = nc.dram_tensor("sc", [D, 1], F32, kind="ExternalInput")
    out_h = nc.dram_tensor("out", [bw, D, N], F16, kind="ExternalOutput")
    import os
    dbg = os.environ.get("BASS_DBG", "")
    dbg_h = None
    if dbg:
        dbg_h = nc.dram_tensor("dbg", [MSG_H, 8192], F16, kind="ExternalOutput")

    with tile.TileContext(nc) as tc:
        with (
            tc.tile_pool(name="consts", bufs=1) as consts,
            tc.tile_pool(name="build", bufs=1) as build,
            tc.tile_pool(name="rkeep", bufs=1) as rkeep,
            tc.tile_pool(name="feat", bufs=3) as featp,
            tc.tile_pool(name="hpool", bufs=2) as hpool,
            tc.tile_pool(name="mpool", bufs=2) as mpool,
            tc.tile_pool(name="fold", bufs=2) as fold,
            tc.tile_pool(name="sig", bufs=1) as sigp,
            tc.tile_pool(name="xzp", bufs=2) as xzp,
            tc.tile_pool(name="tail", bufs=2) as tailp,
            tc.tile_pool(name="ps", bufs=1, space="PSUM") as ps,
        ):
            # ---- Fbuf zeroing on Pool (idle at start); rows 8..31 of each
            # 32-row quadrant must stay 0 for the K=32 L1 contraction ----
            F_bufs = []
            for fi in range(4):
                Fb = featp.tile([128, GC_COLS], F16, name=f"Fbuf{fi}")
                # F0 zeroed on Vector (fastest start; gates the first
                # feature DMAs), the rest on the otherwise-idle Pool
                (nc.vector if fi == 0 else nc.gpsimd).memset(Fb, 0.0)
                F_bufs.append(Fb)

            # dummy activation: pull the SILU ACT_TABLE_LOAD off the
            # critical path (table loads with the first activation)
            dummy_t = consts.tile([MSG_H, 1], F32)
            nc.vector.memset(dummy_t, 0.0)
            nc.scalar.activation(dummy_t, dummy_t, AF.Silu, bias=0.0)

            # ---- critical constants first (serial sync DMA queue) ----
            w1p_t = consts.tile([128, MSG_H], F16)
            nc.sync.dma_start(out=w1p_t, in_=w1p_h.ap())
            b1_t = consts.tile([MSG_H, 1], F32)
            nc.sync.dma_start(out=b1_t, in_=b1_h.ap())
            # m_i accumulator, col = w*64 + i
            stash_t = consts.tile([MSG_H, npart], F16)

            # ---- stacked r build (fp16), partition = 32*d + w ----
            r2bf = rkeep.tile([32, 2 * GC_COLS], F16, name="r2bf")
            r1bf = rkeep.tile([32, 2 * GC_COLS], F16, name="r1bf")
            xjk = [None, None]

            def emit_build_dmas(c, eng=None):
                # startup (c=0) issues from the Vector DGE so these don't
                # queue behind the serial sync DMA chain
                q = eng if eng is not None else nc.sync
                XI = build.tile([96, GC_COLS], F16, tag="XI",
                                name=f"XI{c}")
                XJc = build.tile([96, NJC], F16, tag="XJc",
                                 name=f"XJc{c}")
                for d in range(D):
                    q.dma_start(
                        out=XI[32 * d:32 * d + 32, :].rearrange(
                            "p (j i) -> p j i", i=N),
                        in_=bass.AP(xh_h, d * N, [[N * D, 32], [0, NJC], [1, N]]),
                    )
                    q.dma_start(
                        out=XJc[32 * d:32 * d + 32, :],
                        in_=bass.AP(xh_h, d * N + NJC * c,
                                    [[N * D, 32], [1, NJC]]),
                    )
                return XI, XJc

            def emit_build_piece(c, XI, XJc, sl, jsl, xjf, tb, tc_, t1, t2):
                # one j-piece of the r build: r2, r1 (Quake rsqrt + Newton)
                xjbc = XJc[:, jsl].unsqueeze(2).broadcast_to(
                    [96, jsl.stop - jsl.start, N])
                nc.vector.tensor_copy(
                    xjf[:, sl].rearrange("p (j i) -> p j i", i=N), xjbc)
                nc.vector.tensor_sub(
                    XI[:, sl].rearrange("p (j i) -> p j i", i=N),
                    XI[:, sl].rearrange("p (j i) -> p j i", i=N), xjbc)
                nc.vector.tensor_mul(XI[:, sl], XI[:, sl], XI[:, sl])
                nc.sync.dma_start(out=tb[:, sl], in_=XI[32:64, sl])
                nc.sync.dma_start(out=tc_[:, sl], in_=XI[64:96, sl])
                nc.vector.tensor_add(tb[:, sl], XI[0:32, sl], tb[:, sl])
                r2c = r2bf[:, GC_COLS * c:GC_COLS * (c + 1)][:, sl]
                r1c = r1bf[:, GC_COLS * c:GC_COLS * (c + 1)][:, sl]
                nc.vector.tensor_add(r2c, tb[:, sl], tc_[:, sl])  # r2
                nc.vector.tensor_scalar(r2c, r2c, 1e-4, None, AO.add)
                nc.vector.tensor_scalar(
                    t1[:, sl].bitcast(U16), r2c.bitcast(U16),
                    1, None, AO.logical_shift_right)
                nc.vector.tensor_scalar(
                    t1[:, sl].bitcast(I16), t1[:, sl].bitcast(I16),
                    0x59b8, -1, AO.subtract, AO.mult)
                nc.vector.tensor_mul(t2[:, sl], t1[:, sl], t1[:, sl])
                nc.vector.tensor_mul(t2[:, sl], t2[:, sl], r2c)
                nc.vector.tensor_scalar(t2[:, sl], t2[:, sl], -0.5, 1.5,
                                        AO.mult, AO.add)
                nc.vector.tensor_mul(t2[:, sl], t1[:, sl], t2[:, sl])
                nc.vector.tensor_mul(r1c, r2c, t2[:, sl])

            def mk_build_tiles(c):
                # scratch tags shared across chunks (chunk 1 reuses chunk 0's)
                xjf = rkeep.tile([96, GC_COLS], F16, name=f"xjf{c}")
                tb = build.tile([32, GC_COLS], F16, tag="tb", name=f"tb{c}")
                tc_ = build.tile([32, GC_COLS], F16, tag="tc", name=f"tc{c}")
                t1 = build.tile([32, GC_COLS], F16, tag="qt1", name=f"qt1{c}")
                t2 = build.tile([32, GC_COLS], F16, tag="qt2", name=f"qt2{c}")
                xjk[c] = xjf
                return xjf, tb, tc_, t1, t2

            def emit_feat_xi_dmas(g, F, eng=None):
                q = eng if eng is not None else nc.sync
                for d in range(D):
                    q.dma_start(
                        out=F[d:128:32, :].rearrange("q (j i) -> q j i", i=N),
                        in_=bass.AP(xh_h, 4 * g * N * D + d * N,
                                    [[N * D, 4], [0, NJC], [1, N]]),
                    )

            def emit_feat_scatters(g, c, F, sl):
                for d in range(D):
                    nc.sync.dma_start(
                        out=F[3 + d:128:32, sl],
                        in_=xjk[c][32 * d + 4 * g:32 * d + 4 * g + 4, sl],
                    )
                nc.sync.dma_start(
                    out=F[6:128:32, sl],
                    in_=r2bf[:, GC_COLS * c:GC_COLS * (c + 1)][4 * g:4 * g + 4, sl])
                nc.sync.dma_start(
                    out=F[7:128:32, sl],
                    in_=r1bf[:, GC_COLS * c:GC_COLS * (c + 1)][4 * g:4 * g + 4, sl])

            # ---- startup: build chunk 0 piece 0, then its feat DMAs.
            # Build DMAs go out on the Scalar DGE, unit-0 xi rows on the
            # Pool DGE (both engines idle), so only w1p/b1 + the piece-0
            # scatters ride the serial sync queue. ----
            XI0, XJc0 = emit_build_dmas(0, eng=nc.scalar)
            F0 = F_bufs[0]
            emit_feat_xi_dmas(0, F0, eng=nc.gpsimd)
            bt0 = mk_build_tiles(0)
            PW0 = GC_COLS // 4
            emit_build_piece(0, XI0, XJc0, slice(0, PW0), slice(0, 8), *bt0)
            emit_feat_scatters(0, 0, F0, slice(0, PW0))
            # w2/b2 needed by L2 of sblock 0 (~15us in)
            w2_t = consts.tile([MSG_H, MSG_H], F16)
            nc.sync.dma_start(out=w2_t, in_=w2_h.ap())
            b2_t = consts.tile([MSG_H, 1], F32)
            nc.sync.dma_start(out=b2_t, in_=b2_h.ap())
            for p in range(1, 4):
                emit_build_piece(0, XI0, XJc0,
                                 slice(PW0 * p, PW0 * (p + 1)),
                                 slice(8 * p, 8 * (p + 1)), *bt0)
                emit_feat_scatters(0, 0, F0, slice(PW0 * p, PW0 * (p + 1)))

            # ---- main stream over 16 (group, chunk) units, 4 sblocks each;
            # ACT2/L2 lag ACT1/L1 by one sblock ----
            gcs = [(g, c) for c in range(NCHUNK) for g in range(NG)]

            def emit_feat_dmas(g, c, idx):
                # called at unit idx-1's top (one-unit prefetch)
                F = F_bufs[idx % 4]
                emit_feat_xi_dmas(g, F)
                emit_feat_scatters(g, c, F, slice(0, GC_COLS))
                return F

            def emit_l1(F, s):
                psA = ps.tile([MSG_H, GC_COLS], F32, tag="A")
                for q in range(4):
                    for mq in range(4):
                        nc.tensor.matmul(
                            psA[32 * mq:32 * (mq + 1), 512 * q:512 * (q + 1)],
                            w1p_t[32 * q:32 * q + 32, 32 * mq:32 * (mq + 1)],
                            F[32 * q:32 * q + 32, 512 * s:512 * (s + 1)],
                            start=True, stop=True,
                            tile_position=(32 * q, 32 * mq),
                        )
                return psA

            # DVE bit-trick silu tiles
            sA = sigp.tile([MSG_H, GC_COLS], F16, name="sigA")
            sB = sigp.tile([MSG_H, GC_COLS], F16, name="sigB")
            sC = sigp.tile([MSG_H, GC_COLS], F16, name="sigC")

            def emit_silu2_head(psB):
                # Single PSUM reader: xz = fp16(psB + b2) -- the bias that
                # ACT2 would have applied.  Emitted with high priority so
                # the Vector queue reaches it promptly (otherwise psB blocks
                # the next sblock's L2 behind the multi-us chain backlog).
                xz = xzp.tile([MSG_H, GC_COLS], F16, tag="xz")
                with tc.high_priority(offset=260):
                    nc.vector.tensor_scalar(xz, psB, b2_t, None, AO.add)
                return xz

            def emit_silu2_chain(xz, ms):
                nc.vector.tensor_scalar(sA, xz, S_CLO, S_CHI, AO.max, AO.min)
                nc.vector.tensor_scalar(sB.bitcast(I16), sA, -S_ES, S_EO,
                                        AO.mult, AO.add)
                nc.vector.tensor_scalar(sA, sB, S_DS, 1.0, AO.mult, AO.add)
                nc.vector.tensor_scalar(sB.bitcast(I16), sA.bitcast(I16),
                                        S_MG, -1, AO.subtract, AO.mult)
                nc.vector.tensor_mul(sC, sA, sB)        # t = d*r0
                nc.vector.tensor_scalar(sC, sC, -1.0, 2.0, AO.mult, AO.add)
                nc.vector.tensor_mul(sB, sB, sC)        # r = r0*(2-t)
                nc.vector.tensor_mul(ms, xz, sB)        # m = x*r

            def emit_l2_matmuls(h, s):
                psB = ps.tile([MSG_H, GC_COLS], F32, tag="B")
                for k in range(4):
                    nc.tensor.matmul(
                        psB[:, 512 * k:512 * (k + 1)],
                        w2_t,
                        h[:, 2048 * s + 512 * k:2048 * s + 512 * (k + 1)],
                        start=True, stop=True,
                    )
                return psB

            def emit_fold_piece(g, c, s, m, v3, gsb):
                # zero self-messages: within sblock s cols = 512q + 64jj + i,
                # diag at i = 32c + 8s + jj -> col = 512q + 65jj + 32c + 8s
                ms = m[:, 2048 * s:2048 * (s + 1)]
                v = ms.rearrange("p (q b) -> p q b", q=4)
                nc.gpsimd.memset(v[:, :, 32 * c + 8 * s::65], 0.0)
                eng = nc.gpsimd if gsb in POOL_FOLD else nc.vector
                # 3-level pairwise tree over the 8 j's, in place in m
                m3 = ms.rearrange("p (q j i) -> p q j i", q=4, j=8)
                eng.tensor_add(m3[:, :, 0:4, :], m3[:, :, 0:4, :],
                               m3[:, :, 4:8, :])
                eng.tensor_add(m3[:, :, 0:2, :], m3[:, :, 0:2, :],
                               m3[:, :, 2:4, :])
                eng.tensor_add(
                    v3[:, 256 * s:256 * (s + 1)].rearrange(
                        "p (q i) -> p q i", q=4),
                    m3[:, :, 0, :], m3[:, :, 1, :])

            def emit_fold_final(g, c, v3):
                # cross-sblock + stash adds on Pool (otherwise idle)
                ta = fold.tile([MSG_H, 256], F16, tag="ta")
                nc.gpsimd.tensor_add(ta, v3[:, 0:256], v3[:, 256:512])
                tb2 = fold.tile([MSG_H, 256], F16, tag="tb2")
                nc.gpsimd.tensor_add(tb2, v3[:, 512:768], v3[:, 768:1024])
                sl = stash_t[:, 256 * g:256 * (g + 1)]
                if c == 0:
                    nc.gpsimd.tensor_add(sl, ta, tb2)
                else:
                    tc2 = fold.tile([MSG_H, 256], F16, tag="tc2")
                    nc.gpsimd.tensor_add(tc2, ta, tb2)
                    nc.gpsimd.tensor_add(sl, sl, tc2)

            psi_consts = {}

            def emit_psi_consts():
                pw1m_t = consts.tile([MSG_H, HID], F16)
                nc.sync.dma_start(out=pw1m_t, in_=pw1m_h.ap())
                pw1x_t = consts.tile([D, HID], F16)
                nc.sync.dma_start(out=pw1x_t, in_=pw1x_h.ap())
                pw2_t = consts.tile([HID, HID], F16)
                nc.sync.dma_start(out=pw2_t, in_=pw2_h.ap())
                pw3_t = consts.tile([HID, D], F16)
                nc.sync.dma_start(out=pw3_t, in_=pw3_h.ap())
                pb1_t = consts.tile([HID, 1], F32)
                nc.sync.dma_start(out=pb1_t, in_=pb1_h.ap())
                pb2_t = consts.tile([HID, 1], F32)
                nc.sync.dma_start(out=pb2_t, in_=pb2_h.ap())
                pb3_t = consts.tile([D, 1], F32)
                nc.sync.dma_start(out=pb3_t, in_=pb3_h.ap())
                sc_t = consts.tile([D, 1], F32)
                nc.sync.dma_start(out=sc_t, in_=sc_h.ap())
                xT_t = consts.tile([D, npart], F16)
                nc.sync.dma_start(
                    out=xT_t,
                    in_=bass.AP(xh_h, 0, [[N, D], [N * D, bw], [1, N]]),
                )
                psi_consts.update(pw1m=pw1m_t, pw1x=pw1x_t, pw2=pw2_t,
                                  pw3=pw3_t, pb1=pb1_t, pb2=pb2_t, pb3=pb3_t,
                                  sc=sc_t, xT=xT_t)

            unit_info = {}  # (g, c) -> {'v3': tile, 'done': set()}
            pending = []    # deferred DVE chains: (g, c, s, m, xz, slot)

            def fold_and_maybe_final(g, c, s, m, gsb):
                ui = unit_info[(g, c)]
                emit_fold_piece(g, c, s, m, ui['v3'], gsb)
                ui['done'].add(s)
                if len(ui['done']) == 4:
                    emit_fold_final(g, c, ui['v3'])

            def flush_pending(before_slot):
                while pending and pending[0][5] < before_slot:
                    fg, fc, fs, fm, fxz, _ = pending.pop(0)
                    emit_silu2_chain(fxz, fm[:, 2048 * fs:2048 * (fs + 1)])
                    fold_and_maybe_final(fg, fc, fs, fm,
                                         4 * (fc * NG + fg) + fs)

            prev = None  # (g, c, h, m, s) of the lagging sblock stream
            slot = 0
            for idx, (g, c) in enumerate(gcs):
                if idx == 1:
                    # chunk-1 r build mid-stream (DVE has slack early)
                    XI1, XJc1 = emit_build_dmas(1)
                    bt1 = mk_build_tiles(1)
                    for p in range(2):
                        emit_build_piece(1, XI1, XJc1,
                                         slice(1024 * p, 1024 * (p + 1)),
                                         slice(16 * p, 16 * (p + 1)), *bt1)
                if idx == 4:
                    emit_psi_consts()
                # one-unit feature prefetch: emit unit idx+1's DMAs now
                if idx + 1 < len(gcs):
                    ng_, nc_ = gcs[idx + 1]
                    emit_feat_dmas(ng_, nc_, idx + 1)
                F = F_bufs[idx % 4]
                h = hpool.tile([MSG_H, 4 * GC_COLS], F16, tag="h",
                               name=f"h_{g}_{c}")
                m = mpool.tile([MSG_H, 4 * GC_COLS], F16, tag="m",
                               name=f"m_{g}_{c}")
                v3u = fold.tile([MSG_H, 1024], F16, tag="v3",
                                name=f"v3_{g}_{c}")
                unit_info[(g, c)] = {'v3': v3u, 'done': set()}
                for s in range(4):
                    flush_pending(slot)
                    pl2 = prev
                    psA = emit_l1(F, s)
                    nc.scalar.activation(h[:, 2048 * s:2048 * (s + 1)], psA,
                                         AF.Silu, bias=b1_t, scale=1.0)
                    if pl2 is not None:
                        pg, pc, ph, pm, psb = pl2
                        pgsb = 4 * (pc * NG + pg) + psb
                        psB = emit_l2_matmuls(ph, psb)
                        if pgsb in DVE_SB:
                            xz = emit_silu2_head(psB)
                            pending.append((pg, pc, psb, pm, xz, slot))
                        else:
                            nc.scalar.activation(
                                pm[:, 2048 * psb:2048 * (psb + 1)], psB,
                                AF.Silu, bias=b2_t, scale=1.0)
                            fold_and_maybe_final(pg, pc, psb, pm, pgsb)
                    prev = (g, c, h, m, s)
                    slot += 1
                if dbg == "h0" and (g, c) == (0, 0):
                    nc.sync.dma_start(out=bass.AP(dbg_h, 0, [[8192, MSG_H], [1, 8192]]), in_=h)
            # drain the lagging sblock + deferred chains
            pg, pc, ph, pm, psb = prev
            pgsb = 4 * (pc * NG + pg) + psb
            psB = emit_l2_matmuls(ph, psb)
            if pgsb in DVE_SB:
                xz = emit_silu2_head(psB)
                pending.append((pg, pc, psb, pm, xz, slot))
            else:
                nc.scalar.activation(pm[:, 2048 * psb:2048 * (psb + 1)], psB,
                                     AF.Silu, bias=b2_t, scale=1.0)
                fold_and_maybe_final(pg, pc, psb, pm, pgsb)
            flush_pending(slot + 1)
            if dbg == "m0":
                nc.sync.dma_start(
                    out=bass.AP(dbg_h, 0, [[8192, MSG_H], [1, 8192]]), in_=pm)
            if dbg == "stash":
                nc.sync.dma_start(
                    out=bass.AP(dbg_h, 0, [[8192, MSG_H], [1, 2048]]),
                    in_=stash_t)

            # ---- psi MLP tail, two 1024-col halves pipelined ----
            pc_ = psi_consts
            for hf in range(2):
                sl = slice(1024 * hf, 1024 * (hf + 1))
                u1 = tailp.tile([HID, 1024], F16, tag="u1")
                u2 = tailp.tile([HID, 1024], F16, tag="u2")
                dxs = tailp.tile([D, 1024], F16, tag="dxs")
                psA = ps.tile([HID, 1024], F32, tag="A")
                for s in range(2):
                    ssl = slice(1024 * hf + 512 * s, 1024 * hf + 512 * (s + 1))
                    osl = slice(512 * s, 512 * (s + 1))
                    nc.tensor.matmul(psA[:, osl], pc_["pw1m"], stash_t[:, ssl],
                                     start=True, stop=False)
                    nc.tensor.matmul(psA[:, osl], pc_["pw1x"], pc_["xT"][:, ssl],
                                     start=False, stop=True)
                nc.scalar.activation(u1, psA, AF.Silu, bias=pc_["pb1"], scale=1.0)
                psB = ps.tile([HID, 1024], F32, tag="B")
                for s in range(2):
                    osl = slice(512 * s, 512 * (s + 1))
                    nc.tensor.matmul(psB[:, osl], pc_["pw2"], u1[:, osl],
                                     start=True, stop=True)
                nc.scalar.activation(u2, psB, AF.Silu, bias=pc_["pb2"], scale=1.0)
                psD = ps.tile([D, 1024], F32, tag="A")
                for s in range(2):
                    osl = slice(512 * s, 512 * (s + 1))
                    nc.tensor.matmul(psD[:, osl], pc_["pw3"], u2[:, osl],
                                     start=True, stop=True)
                nc.scalar.activation(dxs, psD, AF.Tanh, bias=pc_["pb3"], scale=1.0)
                nc.vector.tensor_scalar_mul(dxs, dxs, pc_["sc"])
                nc.sync.dma_start(
                    out=bass.AP(out_h, 16 * hf * N * D,
                                [[N, D], [N * D, bw // 2], [1, N]]),
                    in_=dxs.rearrange("p (w i) -> p w i", i=N),
                )

    nc.compile()
    return nc


def host_inputs(x, phi_w1, phi_b1, phi_w2, phi_b2,
                psi_w1, psi_b1, psi_w2, psi_b2, psi_w3, psi_b3, bf_scale,
                bw=BW, ncores=NCORES):
    """Per-core in_maps from the full problem inputs."""
    F16 = np.float16
    x = np.asarray(x, np.float32)
    w1 = np.asarray(phi_w1, np.float64)
    w1p = np.concatenate([
        w1[0:3] + w1[6:9],      # xi rows (r folded in)
        w1[3:6] - w1[6:9],      # xj rows
        w1[10:11],              # r2
        w1[9:10],               # r1
    ], axis=0)
    sc = np.maximum(np.float32(bf_scale), 0.0)
    w1p4 = np.zeros((128, MSG_H), np.float64)
    for q in range(4):
        w1p4[32 * q:32 * q + 8] = w1p
    const = {
        "w1p": w1p4.astype(F16),
        "b1": np.asarray(phi_b1, np.float32).reshape(MSG_H, 1),
        "w2": np.asarray(phi_w2, F16),
        "b2": np.asarray(phi_b2, np.float32).reshape(MSG_H, 1),
        "pw1x": np.ascontiguousarray(np.asarray(psi_w1, F16)[0:3]),
        "pw1m": np.ascontiguousarray(np.asarray(psi_w1, F16)[3:]),
        "pb1": np.asarray(psi_b1, np.float32).reshape(HID, 1),
        "pw2": np.asarray(psi_w2, F16),
        "pb2": np.asarray(psi_b2, np.float32).reshape(HID, 1),
        "pw3": np.asarray(psi_w3, F16),
        "pb3": np.asarray(psi_b3, np.float32).reshape(D, 1),
        "sc": np.full((D, 1), sc, np.float32),
    }
    in_maps = []
    for core in range(ncores):
        xs = np.ascontiguousarray(
            x[core * bw:(core + 1) * bw].transpose(0, 2, 1))
        in_maps.append({"xh": xs.astype(F16), **const})
    return in_maps


_cached_nc = None
LAST_EXEC_NS = None
LAST_PROFILE_JSON = None
LAST_TRACE_PATH = None


def kernel(x, spin, phi_w1, phi_b1, phi_w2, phi_b2,
           psi_w1, psi_b1, psi_w2, psi_b2, psi_w3, psi_b3, bf_scale):
    global _cached_nc
    from concourse.bass_utils import run_bass_kernel_spmd

    if _cached_nc is None:
        _cached_nc = build_program()
    in_maps = host_inputs(x, phi_w1, phi_b1, phi_w2, phi_b2,
                          psi_w1, psi_b1, psi_w2, psi_b2, psi_w3, psi_b3,
                          bf_scale)
    import os
    trace = bool(os.environ.get("BASS_TRACE"))
    res = run_bass_kernel_spmd(_cached_nc, in_maps, core_ids=list(range(NCORES)),
                               trace=trace)
    global LAST_EXEC_NS, LAST_PROFILE_JSON, LAST_TRACE_PATH
    if res.exec_time_ns is not None:
        LAST_EXEC_NS = res.exec_time_ns
    if res.profile_json is not None:
        LAST_PROFILE_JSON = res.profile_json
    if res.instructions_and_trace is not None:
        LAST_TRACE_PATH = res.instructions_and_trace[1]
    out = np.concatenate(
        [r["out"].transpose(0, 2, 1) for r in res.results], axis=0)
    return out.astype(np.float32)
